# revision 6
# baseline (speedup 1.0000x reference)
"""Trainium2 Bass kernel for nn_AutoregressiveNetwork (MADE-style dense MLP).

Data-parallel over the batch: 8 NeuronCores, 2048 rows each. All 64
per-dimension subnetworks (net0 folded in as {W1=0, B1=w0[0]+b0, W2=I,
W3=v0, B3=c0}) run in feature-major layout (x.T on SBUF partitions):

  L1 (fp16): two concurrent row-tiled matmuls, each K=64 / M=128
      computing two nets at once (nets share the streamed x.T); the L1
      bias rides a ones-row of xT2 (x col 63 is dead under the mask).
  L2 (fp16): block-diagonal K=128 / M=128 matmuls, two nets per
      instruction.
  L3 (fp16): two-net block-diag K=128 / M=32 (zero-padded) matmuls,
      col-tiled across the four 32-column groups -> four (net, scale/
      trans) rows per group land in one PSUM bank.

fp16 streams one column/cycle (4x faster than fp32) and its weights are
fast-weight-load eligible; PSUM accumulation stays fp32. The structure
(dst partition base 0 everywhere except the col-tiled L3) was shaped by
an earlier float32r version and kept.

The kernel is bound by PSUM evacuation: 288 bank-reads/core through the
only two engines with PSUM ports (ScalarE ~431ns per 512-col fp32 bank,
VectorE ~533ns; PE writes PSUM fp32-only, DMA/GPSIMD have no PSUM path).
The relu/bias evacuations are split to balance the engines at their
streaming rates (ScalarE 160 ops ~ VectorE 128 ops ~ 68-69us):
  - ScalarE: all L1 relus, plus the even-group L2 evacs (Relu+bias),
    each DEFERRED one group so its L2-matmul dependency is settled
    before it reaches ScalarE's strict-FIFO queue head (the deferred
    op's L3-matmul consumer moves with it).
  - VectorE: odd-group L2 evacs and ALL L3 window evacs. Measured:
    ScalarE reads of the 4-writer L3 PSUM banks cost ~1us/op (mechanism
    unidentified; single-writer banks read at full rate), so P3 readers
    stay on VectorE. This split measured ~9% faster than the previous
    5/8-3/8 L3 split in same-round A/B (72.8us vs 80.2us).
"""
import numpy as np

from concourse import bacc, tile, mybir
from concourse.bass_utils import run_bass_kernel_spmd

DIM = 64
HID = 64
BATCH = 16384
NCORES = 8
BL = BATCH // NCORES          # 2048 batch rows per core
NT = 512                      # free-dim per matmul (one fp32 PSUM bank)
T = BL // NT                  # batch tiles per core
G = 16                        # groups of 4 nets
F32 = mybir.dt.float32
F32R = mybir.dt.float32r
F16 = mybir.dt.float16

TRACE = False                 # no NTFF hook in this container
_cache = {}


def _build(reps=1):
    key = ("nc", reps)
    if key in _cache:
        return _cache[key]
    nc = bacc.Bacc("TRN2", target_bir_lowering=False, debug=False,
                   num_devices=NCORES)

    xT2 = nc.declare_dram_parameter("xT2", [128, BL], F16, isOutput=False)
    lw1 = nc.declare_dram_parameter("lw1", [128, G * 128], F16, isOutput=False)
    lw2 = nc.declare_dram_parameter("lw2", [128, G * 256], F16, isOutput=False)
    lw3 = nc.declare_dram_parameter("lw3", [128, 32 * 32], F16, isOutput=False)
    bb2 = nc.declare_dram_parameter("bb2", [128, 2 * G], F32, isOutput=False)
    bb3 = nc.declare_dram_parameter("bb3", [128, 1], F32, isOutput=False)
    out = nc.declare_dram_parameter("out", [128, BL], F32, isOutput=True)

    Relu = mybir.ActivationFunctionType.Relu
    ADD = mybir.AluOpType.add
    MAX = mybir.AluOpType.max

    with tile.TileContext(nc) as tc:
        with (
            tc.tile_pool(name="const", bufs=1) as cpool,
            tc.tile_pool(name="act", bufs=6) as apool,
            tc.tile_pool(name="ps1", bufs=2, space="PSUM") as pspool1,
            tc.tile_pool(name="ps2", bufs=3, space="PSUM") as pspool2,
            tc.tile_pool(name="ps3", bufs=1, space="PSUM") as pspool3,
        ):
            xw = cpool.tile([128, BL], F16)
            w1 = cpool.tile([128, G * 128], F16)
            w2 = cpool.tile([128, G * 256], F16)
            w3 = cpool.tile([128, 32 * 32], F16)
            b2 = cpool.tile([128, 2 * G], F32)
            b3 = cpool.tile([128, 1], F32)
            osb = cpool.tile([128, BL], F32)
            # chunked loads so group-0 compute starts after ~1/8 of the
            # weights have landed
            for sb, dr in ((b2, bb2), (b3, bb3), (w3, lw3)):
                nc.sync.dma_start(sb[:], dr[:])
            for i in range(8):
                nc.sync.dma_start(xw[:, i * 256:(i + 1) * 256],
                                  xT2[:, i * 256:(i + 1) * 256])
                nc.sync.dma_start(w1[:, i * 256:(i + 1) * 256],
                                  lw1[:, i * 256:(i + 1) * 256])
                nc.sync.dma_start(w2[:, i * 512:(i + 1) * 512],
                                  lw2[:, i * 512:(i + 1) * 512])

            for _rep in range(reps):
              pend_l2 = None
              for t in range(T):
                xs = xw[:, t * NT:(t + 1) * NT]
                # one shared L3 accumulation bank per batch tile: chunk c
                # (cols 32c..32c+32) accumulates all 8 pairs' outputs, pair q
                # landing on rows 32c+4q..32c+4q+4 via col-shifted weights
                # (zero cols elsewhere accumulate harmlessly). start resets
                # the chunk on pair 0, stop closes the group on pair 7.
                P3 = pspool3.tile([128, NT], F32, tag="ps3")

                def l3(c, q, h2t):
                    nc.tensor.matmul(P3[32 * c:32 * c + 32, :],
                                     w3[:, q * 128 + c * 32:q * 128 + c * 32 + 32],
                                     h2t[:], tile_position=(0, 32 * c),
                                     start=(q == 0), stop=(q == 7))

                for g in range(G):
                    q, gg = g // 2, g % 2
                    c1 = g * 128
                    c2 = g * 256

                    # ---- L1: h1 = relu(x @ W1m + b1); bias rides the
                    # ones-row of xT2 (x col 63 is unused by every net), so
                    # the evacuation is a bias-free relu over both banks
                    P1 = pspool1.tile([128, 2 * NT], F32, tag="ps1")
                    nc.tensor.matmul(P1[:, 0:NT], w1[0:64, c1:c1 + 128],
                                     xs[0:64, :], tile_position=(0, 0))
                    nc.tensor.matmul(P1[:, NT:2 * NT], w1[64:128, c1:c1 + 128],
                                     xs[64:128, :], tile_position=(64, 0))

                    s12 = apool.tile([128, 2 * NT], F16, tag="s12")
                    # two FD-512 ops beat one FD-1024 on ScalarE (measured
                    # 2x455ns vs 1037ns - the activation pipe is super-linear
                    # in free-dim length)
                    nc.scalar.activation(s12[:, 0:NT], P1[:, 0:NT], Relu,
                                         bias=0.0)
                    nc.scalar.activation(s12[:, NT:2 * NT], P1[:, NT:2 * NT],
                                         Relu, bias=0.0)

                    if gg == 1 and pend_l2 is not None:
                        dt1, dP2a, dbc = pend_l2
                        nc.scalar.activation(dt1[:], dP2a[:], Relu,
                                             bias=b2[:, dbc:dbc + 1])
                        l3(0, q, dt1)
                        pend_l2 = None

                    # ---- L2: h2 = relu(h1 @ W2 + b2), block-diag 2 nets
                    P2a = pspool2.tile([128, NT], F32, tag="ps2")
                    P2b = pspool2.tile([128, NT], F32, tag="ps2")
                    nc.tensor.matmul(P2a[:], w2[:, c2:c2 + 128], s12[:, 0:NT],
                                     tile_position=(0, 0))
                    nc.tensor.matmul(P2b[:], w2[:, c2 + 128:c2 + 256],
                                     s12[:, NT:2 * NT], tile_position=(0, 0))

                    t1 = apool.tile([128, NT], F16, tag="t1")
                    t2 = apool.tile([128, NT], F16, tag="t2")
                    if gg == 0 and q % 2 == 0:
                        # defer t1's evac to ScalarE, emitted after the next
                        # group's s12 ops (clears the L2-matmul dependency
                        # without head-of-line blocking ScalarE's queue);
                        # its L3 matmul consumer is deferred with it.
                        # ScalarE tolerates single-writer P2 bank reads at
                        # full rate; multi-writer P3 banks are DVE-only
                        # (those cost ~1us/op on ScalarE). Only every other
                        # even group defers to ScalarE - the rest go to
                        # VectorE to balance the two engines' evac loads
                        # (ScalarE 36 ops/tile @431ns ~ VectorE 29 @533ns).
                        pend_l2 = (t1, P2a, 2 * g)
                    else:
                        nc.vector.tensor_scalar(t1[:], P2a[:],
                                                b2[:, 2 * g:2 * g + 1],
                                                0.0, ADD, MAX)
                    nc.vector.tensor_scalar(t2[:], P2b[:], b2[:, 2 * g + 1:2 * g + 2],
                                            0.0, ADD, MAX)

                    # ---- L3: out = h2 @ W3 (bias added at the single
                    # per-tile evac). Chunks 0..3 <- (t1,t2) of even group,
                    # (t1,t2) of odd group; adjacent different-chunk matmuls
                    # overlap via PE column tiling.
                    if gg == 0:
                        if q % 2 == 1:
                            l3(0, q, t1)
                        l3(1, q, t2)
                    else:
                        l3(2, q, t1)
                        l3(3, q, t2)

                # single evacuation of the packed L3 bank (all 64 nets'
                # scale/trans rows for this batch tile) + per-row bias
                off = t * NT
                nc.vector.tensor_scalar(osb[:, off:off + NT], P3[:],
                                        b3[:, 0:1], None, ADD)
                if _rep == reps - 1:
                    nc.sync.dma_start(out[:, off:off + NT],
                                      osb[:, off:off + NT])


    nc.compile()
    _cache[key] = nc
    return nc


def _pair_of(q, c):
    g = 2 * q + c // 2
    return (4 * g, 4 * g + 1) if c % 2 == 0 else (4 * g + 2, 4 * g + 3)


def _pack_weights(w0, b0, v0, c0, W1, B1, W2, B2, W3, B3):
    f = np.float32
    # 64 nets in device order; net 0 is the constant network.
    W1n = np.zeros((64, DIM, HID), f)
    B1n = np.zeros((64, HID), f)
    W2n = np.zeros((64, HID, HID), f)
    B2n = np.zeros((64, HID), f)
    W3n = np.zeros((64, HID, 2), f)
    B3n = np.zeros((64, 2), f)

    mask = (np.arange(DIM)[None, :] < np.arange(1, DIM)[:, None]).astype(f)
    W1n[1:] = W1 * mask[:, :, None]
    B1n[1:] = B1
    W2n[1:] = W2
    B2n[1:] = B2
    W3n[1:] = W3
    B3n[1:] = B3
    # net 0: Linear(1,H)->ReLU->Linear(H,2) with constant ones input
    B1n[0] = w0[0] + b0
    W2n[0] = np.eye(HID, dtype=f)
    W3n[0] = v0
    B3n[0] = c0

    lw1 = np.zeros((128, G * 128), np.float16)
    lw2 = np.zeros((128, G * 256), np.float16)
    bb2 = np.zeros((128, 2 * G), f)
    for g in range(G):
        n = 4 * g
        c1 = g * 128
        c2 = g * 256
        # L1: [W1 n | W1 n+1] on partitions 0-63, [W1 n+2 | W1 n+3] on
        # 64-127; W1 row 63 is zero for every net (autoregressive mask), so
        # it carries the L1 bias against the ones-row of xT2
        lw1[0:64, c1:c1 + 64] = W1n[n]
        lw1[0:64, c1 + 64:c1 + 128] = W1n[n + 1]
        lw1[64:128, c1:c1 + 64] = W1n[n + 2]
        lw1[64:128, c1 + 64:c1 + 128] = W1n[n + 3]
        lw1[63, c1:c1 + 128] = np.concatenate([B1n[n], B1n[n + 1]])
        lw1[127, c1:c1 + 128] = np.concatenate([B1n[n + 2], B1n[n + 3]])
        # L2: block-diag pairs
        lw2[0:64, c2:c2 + 64] = W2n[n]
        lw2[64:128, c2 + 64:c2 + 128] = W2n[n + 1]
        lw2[0:64, c2 + 128:c2 + 192] = W2n[n + 2]
        lw2[64:128, c2 + 192:c2 + 256] = W2n[n + 3]
        bb2[:, 2 * g] = np.concatenate([B2n[n], B2n[n + 1]])
        bb2[:, 2 * g + 1] = np.concatenate([B2n[n + 2], B2n[n + 3]])

    # L3 weight tile (q, c) is 32 wide with its 4 data columns at offset 4q,
    # so pair q's outputs land on PSUM rows 32c+4q..32c+4q+4 of the shared
    # accumulation bank (zero columns accumulate harmlessly elsewhere).
    lw3 = np.zeros((128, 32 * 32), np.float16)
    bb3 = np.zeros((128, 1), f)
    for q in range(8):
        for c in range(4):
            p0, p1 = _pair_of(q, c)
            col = q * 128 + c * 32 + 4 * q
            lw3[0:64, col:col + 2] = W3n[p0]
            lw3[64:128, col + 2:col + 4] = W3n[p1]
            bb3[32 * c + 4 * q:32 * c + 4 * q + 4, 0] = [
                B3n[p0, 0], B3n[p0, 1], B3n[p1, 0], B3n[p1, 1]]
    return dict(lw1=lw1, lw2=lw2, lw3=lw3, bb2=bb2, bb3=bb3)


def _unshard_core(oc, scales, trans, r0):
    """Scatter one core's [128, BL] output block into scales/trans rows
    r0:r0+BL. Partition 32c + 4q + r holds (net, out) = _pair_of(q, c)
    expanded as [p0 scale, p0 trans, p1 scale, p1 trans]."""
    for q in range(8):
        for c in range(4):
            p0, p1 = _pair_of(q, c)
            base = 32 * c + 4 * q
            scales[r0:r0 + BL, p0] = oc[base + 0]
            trans[r0:r0 + BL, p0] = oc[base + 1]
            scales[r0:r0 + BL, p1] = oc[base + 2]
            trans[r0:r0 + BL, p1] = oc[base + 3]


def kernel(x, w0, b0, v0, c0, W1, B1, W2, B2, W3, B3):
    x = np.asarray(x, np.float32)
    args = [np.asarray(a, np.float32) for a in (w0, b0, v0, c0, W1, B1, W2, B2,
                                                W3, B3)]
    wdict = _pack_weights(*args)

    nc = _build()
    in_maps = []
    for core in range(NCORES):
        xT = np.ascontiguousarray(x[core * BL:(core + 1) * BL].T)   # [64, BL]
        xT2 = np.concatenate([xT, xT], axis=0)
        xT2[63, :] = 1.0          # ones-row carries the L1 bias
        xT2[127, :] = 1.0
        in_maps.append({"xT2": xT2.astype(np.float16), **wdict})

    res = run_bass_kernel_spmd(nc, in_maps, core_ids=list(range(NCORES)),
                               trace=TRACE)
    kernel.last_exec_time_ns = res.exec_time_ns

    scales = np.empty((BATCH, DIM), np.float32)
    trans = np.empty((BATCH, DIM), np.float32)
    for core in range(NCORES):
        _unshard_core(res.results[core]["out"], scales, trans, core * BL)

    np.clip(scales, -5.0, 5.0, out=scales)
    return scales, trans

